# revision 32
# baseline (speedup 1.0000x reference)
"""Trainium2 Bass kernel for nn_CrossHeadAttention.

Computation (per batch b):
  pooled = mean(x[b], spatial)                       # (NH, CH)
  aw     = tiny transformer block on pooled          # (NH, CH)
  out[b] = x[b] * (1 + aw)[..., None, None]

Memory-bound: 256 MiB in + 256 MiB out, data-parallel over batch
(32 batches -> 8 cores x 4 batches). Per core, each batch's
(4, 8, 256, 256) slab is one [128, 16384] SBUF tile
(partition = head*32 + ch*4 + spatial_quarter).

Schedule: per batch, strictly
  [chunked loads -> spatial-sum passes -> tiny math -> multiplies -> stores]
Loads ride the sync HWDGE queue, stores the scalar HWDGE queue -- two
independent rings round-robined by the 16 SDMA engines.

Engine balance (the per-core elementwise work is 8.4M-elem reduce +
8.4M-elem multiply; DVE alone would be the pipeline bottleneck):
  - spatial sums: 8 sub-chunks/batch, alternating ACT (in-place Copy with
    accum_out) and DVE (in-place tensor_scalar x1.0 with accum_out)
  - multiplies: 1 chunk on ACT (Copy w/ scale), 3 on DVE
  - tiny-math: PE matmuls; layernorm rstd = bitcast fast-inverse-sqrt
    (int32 shift/xor/add + 2 Newton steps, max rel err 5e-6) on DVE;
    gelu via tanh form 0.5*u*(1+tanh(z)) with ACT Tanh
ACT functions used: Exp, Tanh, Copy, Square -- ALL in activation-table
set 0 (exp_and_others), so exactly one ACT_TABLE_LOAD (warmed at t=0).
(Ln would pull set 5 and re-trigger ~2.7us table switches per use: the
bass inserter maps each function to the first set containing it.)

All small parameters ship in ONE packed [128, CF] DMA on the scalar queue.
Sub-chunked sums keep any scheduler gap-filling quanta small (~1-2us), so
batch b's store-critical math is never stuck behind a 4.4us younger-batch
reduce on an in-order engine.
"""

from contextlib import ExitStack

import numpy as np

import concourse.bacc as bacc
import concourse.bass as bass
import concourse.tile as tile
from concourse import mybir

NCORES = 8
B, NH, CH = 32, 4, 8
H = W = 256
S = H * W                  # spatial elements per (b, h, c) plane
HID = 4
BPC = B // NCORES          # batches per core
P = 128                    # SBUF partitions
SPLIT = P // (NH * CH)     # spatial quarters mapped to partitions
FREE = S // SPLIT          # free-dim elements per partition (16384)
NCHUNK = 4                 # load chunks per batch (2 MiB each)
CHUNK = FREE // NCHUNK
NSTORE = 2                 # stores (= half-tiles) per batch (4 MiB each)
STCOLS = FREE // NSTORE
XBUFS = 6                  # half-batch tiles in flight (6 x 4 MiB = 24 MiB)
SCALE = CH ** -0.5
EPS = 1e-5
GK0 = float(np.sqrt(2.0 / np.pi))
GK1 = float(np.sqrt(2.0 / np.pi) * 0.044715)
MAGICP1 = 0x5F3759E0       # fast-rsqrt magic + 1
LOG2E = float(np.log2(np.e))
# cubic fit of 2^f on [-0.5, 1] (covers either f32->i32 rounding mode)
FE0, FE1, FE2, FE3 = 0.99988085, 0.6915571, 0.24245486, 0.06474001
F32 = mybir.dt.float32
I32 = mybir.dt.int32
AFT = mybir.ActivationFunctionType
ALU = mybir.AluOpType
AX = mybir.AxisListType

NRED = 8                   # ACT spatial-sum quanta per batch (2048 cols)
RCOLS = FREE // NRED

# ---- packed constant layout: name -> (rows, col_off, cols) ----
_CONST_LAYOUT = {}
_CF = 0


def _alloc_const(name, rows, cols):
    global _CF
    _CONST_LAYOUT[name] = (rows, _CF, cols)
    _CF += cols


_alloc_const("cmask", P, CH)      # [k, c] = (c(k)==c) / S
_alloc_const("hsel", P, NH)       # [k, h] = (h(k)==h)
_alloc_const("b128", CH, P)       # [c, k] = (c(k)==c)
_alloc_const("ind128", NH, P)     # [h, k] = (h(k)==h)
_alloc_const("wq_t", CH, CH)
_alloc_const("wk_t", CH, CH)
_alloc_const("wv_t", CH, CH)
_alloc_const("wo_t", CH, CH)
_alloc_const("w1_t", CH, HID)
_alloc_const("w2_t", HID, CH)
_alloc_const("eye4", NH, NH)
_alloc_const("bo", NH, CH)        # broadcast rows
_alloc_const("g1", NH, CH)
_alloc_const("beta1", NH, CH)
_alloc_const("g2", NH, CH)
_alloc_const("beta2", NH, CH)
_alloc_const("b1", NH, HID)
_alloc_const("b2", NH, CH)
_alloc_const("gsig", NH, 1)       # sigmoid(gate), host-computed
_alloc_const("omg", NH, 1)        # 1 - sigmoid(gate)
_alloc_const("ones4", NH, 1)
_alloc_const("onesP", P, 1)       # neutral x1.0 token for batch 0's sums
CF = _CF


def _emit(nc, tc, io):
    with ExitStack() as ctx:
        const = ctx.enter_context(tc.tile_pool(name="const", bufs=1))
        xp = ctx.enter_context(tc.tile_pool(name="xp", bufs=XBUFS))
        sm = ctx.enter_context(tc.tile_pool(name="sm", bufs=4))
        ps = ctx.enter_context(tc.tile_pool(name="ps", bufs=8, space="PSUM"))

        const_t = const.tile([P, CF], F32, tag="c_all")
        nc.scalar.dma_start(out=const_t, in_=io["consts"][:])

        def cs(name):
            rows, off, cols = _CONST_LAYOUT[name]
            return const_t[0:rows, off:off + cols]

        cmask, hsel, b128, ind128 = cs("cmask"), cs("hsel"), cs("b128"), cs("ind128")
        wq_t, wk_t, wv_t, wo_t = cs("wq_t"), cs("wk_t"), cs("wv_t"), cs("wo_t")
        w1_t, w2_t, eye4 = cs("w1_t"), cs("w2_t"), cs("eye4")
        bo_bc, g1_bc, beta1_bc = cs("bo"), cs("g1"), cs("beta1")
        g2_bc, beta2_bc = cs("g2"), cs("beta2")
        b1_bc, b2_bc = cs("b1"), cs("b2")
        gsig4, omg4 = cs("gsig"), cs("omg")

        # warm the single ACT table set (exp_and_others) while batch 0 loads
        warm = sm.tile([1, 1], F32, tag="warm")
        nc.vector.memset(warm, 0.0)
        warm2 = sm.tile([1, 1], F32, tag="warm2")
        nc.scalar.activation(out=warm2, in_=warm, func=AFT.Exp)

        def pe_t(src, f, tag):
            # [4, f] -> [f, 4] via PE transpose (fp32 has no DMA transpose)
            tp = ps.tile([f, NH], F32, tag="ps")
            nc.tensor.transpose(tp, src, eye4)
            t = sm.tile([f, NH], F32, tag=tag)
            nc.vector.tensor_copy(out=t, in_=tp)
            return t

        def mm(lhsT, rhs, m, n, tag=None):
            op = ps.tile([m, n], F32, tag="ps")
            nc.tensor.matmul(op, lhsT, rhs, start=True, stop=True)
            if tag is None:
                return op
            t = sm.tile([m, n], F32, tag=tag)
            nc.vector.tensor_copy(out=t, in_=op)
            return t

        def rsqrt4(var, tag):
            # y = 1/sqrt(var + EPS), fast-inverse-sqrt on DVE: bits' =
            # ~(bits >> 1) + (MAGIC+1)  ==  MAGIC - (bits >> 1),
            # then 2 Newton steps y <- y*(1.5 - 0.5*v*y^2).  Max rel 5e-6.
            vpe = sm.tile([NH, 1], F32, tag=tag + "_v")
            nc.vector.tensor_scalar_add(out=vpe, in0=var, scalar1=EPS)
            y = sm.tile([NH, 1], F32, tag=tag + "_y")
            nc.vector.tensor_scalar(out=y[:].bitcast(I32),
                                    in0=vpe[:].bitcast(I32),
                                    scalar1=1, scalar2=-1,
                                    op0=ALU.arith_shift_right,
                                    op1=ALU.bitwise_xor)
            nc.vector.tensor_scalar_add(out=y[:].bitcast(I32),
                                        in0=y[:].bitcast(I32),
                                        scalar1=MAGICP1)
            for it in range(2):
                a = sm.tile([NH, 1], F32, tag=f"{tag}_a{it}")
                nc.vector.tensor_mul(out=a, in0=y, in1=y)
                nc.vector.tensor_mul(out=a, in0=a, in1=vpe)
                nc.vector.tensor_scalar(out=a, in0=a, scalar1=-0.5,
                                        scalar2=1.5, op0=ALU.mult, op1=ALU.add)
                yn = sm.tile([NH, 1], F32, tag=f"{tag}_y{it}")
                nc.vector.tensor_mul(out=yn, in0=y, in1=a)
                y = yn
            return y

        def fexp(src, pre_scale, shape, tag):
            # exp(pre_scale * src) entirely on DVE (bit-trick), so the math
            # chain never crosses the ACT engine (whose in-order queue holds
            # data-starved younger-batch sum quanta): y = src*pre_scale*log2e
            # + 1024; k = int(y); frac = y - k; exp = 2^(k-1024) * poly(frac)
            # with 2^(k-1024) assembled via (k-897)<<23.  Max rel err 7e-4.
            yb = sm.tile(shape, F32, tag=tag + "_yb")
            nc.vector.tensor_scalar(out=yb, in0=src,
                                    scalar1=pre_scale * LOG2E, scalar2=1024.0,
                                    op0=ALU.mult, op1=ALU.add)
            ki = sm.tile(shape, I32, tag=tag + "_ki")
            nc.vector.tensor_copy(out=ki, in_=yb)
            kf = sm.tile(shape, F32, tag=tag + "_kf")
            nc.vector.tensor_copy(out=kf, in_=ki)
            fr = sm.tile(shape, F32, tag=tag + "_fr")
            nc.vector.tensor_sub(out=fr, in0=yb, in1=kf)
            km = sm.tile(shape, I32, tag=tag + "_km")
            nc.vector.tensor_scalar(out=km, in0=ki, scalar1=897, scalar2=None,
                                    op0=ALU.subtract)
            ef = sm.tile(shape, F32, tag=tag + "_ef")
            nc.vector.tensor_scalar(out=ef[:].bitcast(I32), in0=km,
                                    scalar1=23, scalar2=None,
                                    op0=ALU.logical_shift_left)
            p = sm.tile(shape, F32, tag=tag + "_p")
            nc.vector.tensor_scalar(out=p, in0=fr, scalar1=FE3, scalar2=FE2,
                                    op0=ALU.mult, op1=ALU.add)
            nc.vector.tensor_mul(out=p, in0=p, in1=fr)
            nc.vector.tensor_scalar_add(out=p, in0=p, scalar1=FE1)
            nc.vector.tensor_mul(out=p, in0=p, in1=fr)
            nc.vector.tensor_scalar_add(out=p, in0=p, scalar1=FE0)
            out = sm.tile(shape, F32, tag=tag + "_o")
            nc.vector.tensor_mul(out=out, in0=p, in1=ef)
            return out

        def layernorm(src, g_bc, b_bc, tag):
            stats = sm.tile([NH, nc.vector.BN_STATS_DIM], F32, tag=tag + "_st")
            nc.vector.bn_stats(out=stats, in_=src)
            mv = sm.tile([NH, 2], F32, tag=tag + "_mv")
            nc.vector.bn_aggr(out=mv, in_=stats)
            rstd = rsqrt4(mv[:, 1:2], tag)
            xn = sm.tile([NH, CH], F32, tag=tag + "_o")
            nc.vector.tensor_scalar(out=xn, in0=src, scalar1=mv[:, 0:1],
                                    scalar2=rstd, op0=ALU.subtract, op1=ALU.mult)
            nc.vector.tensor_mul(out=xn, in0=xn, in1=g_bc)
            nc.vector.tensor_add(out=xn, in0=xn, in1=b_bc)
            return xn

        def batch(b, token):
            # Two half-batch tiles: finer SBUF reuse (a future batch's loads
            # only wait on one 4 MiB store, not a whole 8 MiB batch), and
            # per-half stores ring as soon as that half's 2 multiplies land.
            #
            # Every spatial-sum op takes `token` (prev batch's mcol rescaled
            # to exactly 1.0) as a neutral multiplicative operand: a REAL
            # data dependency that forbids the list scheduler from placing
            # this batch's 2-4.5us sum quanta ahead of the previous batch's
            # store-critical math/multiply ops on the in-order DVE/ACT
            # engines (in v3 that inversion serialized all stores of batches
            # 1-3 behind the final loads: +30us).
            # sums live ONLY on ACT (small 2048-col quanta); math + all four
            # multiplies live ONLY on DVE (+PE).  The list scheduler then has
            # no sum quantum it could insert between the ~30 small DVE ops of
            # an older batch's store-critical math chain -- only the two tiny
            # ACT crossings (softmax Exp, gelu Tanh) can eat a <2us insertion.
            halves = []
            sums8 = sm.tile([P, NRED], F32, tag="sums8")
            for h in range(NSTORE):
                xh = xp.tile([P, STCOLS], F32, tag="xh")
                for cc in range(NCHUNK // NSTORE):
                    c = h * (NCHUNK // NSTORE) + cc
                    lsl = slice(cc * CHUNK, (cc + 1) * CHUNK)
                    nc.sync.dma_start(out=xh[:, lsl],
                                      in_=io["x"][b][:, c * CHUNK:(c + 1) * CHUNK])
                    for j in range(NRED // NCHUNK):
                        q = c * (NRED // NCHUNK) + j
                        rsl = slice(cc * CHUNK + j * RCOLS,
                                    cc * CHUNK + (j + 1) * RCOLS)
                        nc.scalar.activation(out=xh[:, rsl], in_=xh[:, rsl],
                                             func=AFT.Copy, scale=token,
                                             accum_out=sums8[:, q:q + 1])
                halves.append(xh)

            # spatial mean: fold sub-sums, then fold the partition quarters
            # into pooled [4h, 8c] via selection matmul.
            sums = sm.tile([P, 1], F32, tag="sums")
            nc.vector.reduce_sum(out=sums, in_=sums8, axis=AX.X)
            # next batch's sum-ordering token (sums * 0 + 1 == 1.0): younger
            # sum quanta may never be scheduled ahead of this batch's fold
            ntok = sm.tile([P, 1], F32, tag="ntok")
            nc.vector.tensor_scalar(out=ntok, in0=sums, scalar1=0.0,
                                    scalar2=1.0, op0=ALU.mult, op1=ALU.add)
            csums = sm.tile([P, CH], F32, tag="csums")
            nc.vector.tensor_scalar_mul(out=csums, in0=cmask, scalar1=sums)
            pooled_ps = ps.tile([NH, CH], F32, tag="ps")
            nc.tensor.matmul(pooled_ps, hsel, csums, start=True, stop=True)
            pooled = sm.tile([NH, CH], F32, tag="pooled")
            nc.vector.tensor_copy(out=pooled, in_=pooled_ps)

            xn = layernorm(pooled, g1_bc, beta1_bc, "ln1")
            xnT = pe_t(xn, CH, "xnT")                    # [8, 4]
            qT = mm(wq_t, xnT, CH, NH, "qT")             # [8, 4] = Wq @ xn.T
            kT = mm(wk_t, xnT, CH, NH, "kT")
            v = mm(xnT, wv_t, NH, CH, "v")               # [4, 8] = xn @ Wv.T
            sc = mm(qT, kT, NH, NH)                      # psum [4h, 4g]
            es = fexp(sc, SCALE, [NH, NH], "es")
            rs = sm.tile([NH, 1], F32, tag="rs")
            nc.vector.reduce_sum(out=rs, in_=es, axis=AX.X)
            rr = sm.tile([NH, 1], F32, tag="rr")
            nc.vector.reciprocal(out=rr, in_=rs)
            attn = sm.tile([NH, NH], F32, tag="attn")
            nc.vector.tensor_scalar_mul(out=attn, in0=es, scalar1=rr)
            attnT = pe_t(attn, NH, "attnT")              # [4g, 4h]
            ao = mm(attnT, v, NH, CH, "ao")              # [4, 8] = attn @ V
            aoT = pe_t(ao, CH, "aoT")                    # [8, 4]
            o_ps = mm(aoT, wo_t, NH, CH)                 # psum [4, 8] = ao @ Wo.T
            xat = sm.tile([NH, CH], F32, tag="xat")
            nc.vector.tensor_add(out=xat, in0=o_ps, in1=bo_bc)
            nc.vector.tensor_add(out=xat, in0=xat, in1=pooled)

            xn2 = layernorm(xat, g2_bc, beta2_bc, "ln2")
            xn2T = pe_t(xn2, CH, "xn2T")                 # [8, 4]
            h1_ps = mm(xn2T, w1_t, NH, HID)              # psum [4, 4]
            u = sm.tile([NH, HID], F32, tag="u")
            nc.vector.tensor_add(out=u, in0=h1_ps, in1=b1_bc)
            # tanh-gelu on DVE only: z = GK0*u + GK1*u^3;
            # 0.5*(1+tanh(z)) == 1 - 1/(exp(2z)+1), so gelu = u*(1 - r)
            usq = sm.tile([NH, HID], F32, tag="usq")
            nc.vector.tensor_mul(out=usq, in0=u, in1=u)
            w = sm.tile([NH, HID], F32, tag="w")
            nc.vector.tensor_scalar(out=w, in0=usq, scalar1=GK1, scalar2=GK0,
                                    op0=ALU.mult, op1=ALU.add)
            z = sm.tile([NH, HID], F32, tag="z")
            nc.vector.tensor_mul(out=z, in0=w, in1=u)
            e2z = fexp(z, 2.0, [NH, HID], "e2z")
            ep1 = sm.tile([NH, HID], F32, tag="ep1")
            nc.vector.tensor_scalar_add(out=ep1, in0=e2z, scalar1=1.0)
            rp = sm.tile([NH, HID], F32, tag="rp")
            nc.vector.reciprocal(out=rp, in_=ep1)
            hf = sm.tile([NH, HID], F32, tag="hf")
            nc.vector.tensor_scalar(out=hf, in0=rp, scalar1=-1.0, scalar2=1.0,
                                    op0=ALU.mult, op1=ALU.add)
            h1g = sm.tile([NH, HID], F32, tag="h1g")
            nc.vector.tensor_mul(out=h1g, in0=u, in1=hf)

            h1gT = pe_t(h1g, HID, "h1gT")                # [4hid, 4h]
            f_ps = mm(h1gT, w2_t, NH, CH)                # psum [4, 8]
            xo = sm.tile([NH, CH], F32, tag="xo")
            nc.vector.tensor_add(out=xo, in0=f_ps, in1=b2_bc)
            nc.vector.tensor_add(out=xo, in0=xo, in1=xat)

            # m = 1 + aw = (g * x_out + 1) + (1 - g) * pooled
            d = sm.tile([NH, CH], F32, tag="d")
            nc.vector.tensor_scalar(out=d, in0=xo, scalar1=gsig4,
                                    scalar2=1.0, op0=ALU.mult, op1=ALU.add)
            m4 = sm.tile([NH, CH], F32, tag="m4")
            nc.vector.scalar_tensor_tensor(out=m4, in0=pooled, scalar=omg4,
                                           in1=d, op0=ALU.mult, op1=ALU.add)

            # expand m4 [4h, 8c] -> per-partition scalar mcol [128, 1] with
            # PE only: W128[h, k] = m4[h, c(k)]; mask rows by h(k); column
            # sums distribute the selected value to every partition k.
            m4T = pe_t(m4, CH, "m4T")                    # [8c, 4h]
            w128_ps = ps.tile([NH, P], F32, tag="ps")
            nc.tensor.matmul(w128_ps, m4T, b128, start=True, stop=True)
            v128 = sm.tile([NH, P], F32, tag="v128")
            nc.vector.tensor_mul(out=v128, in0=w128_ps, in1=ind128)
            mcol_ps = ps.tile([P, 1], F32, tag="ps")
            nc.tensor.matmul(mcol_ps, v128, cs("ones4"), start=True, stop=True)
            mcol = sm.tile([P, 1], F32, tag="mcol")
            nc.vector.tensor_copy(out=mcol, in_=mcol_ps)

            # multiply in place (all DVE) + stores; the last batch stores
            # per-chunk so its tail drain starts right after the first mult
            last = b == BPC - 1
            for h in range(NSTORE):
                xh = halves[h]
                for cc in range(NCHUNK // NSTORE):
                    lsl = slice(cc * CHUNK, (cc + 1) * CHUNK)
                    nc.vector.tensor_scalar_mul(out=xh[:, lsl],
                                                in0=xh[:, lsl], scalar1=mcol)
                    if last:
                        c = h * (NCHUNK // NSTORE) + cc
                        nc.scalar.dma_start(
                            out=io["y"][b][:, c * CHUNK:(c + 1) * CHUNK],
                            in_=xh[:, lsl])
                if not last:
                    ssl = slice(h * STCOLS, (h + 1) * STCOLS)
                    nc.scalar.dma_start(out=io["y"][b][:, ssl], in_=xh)
            return ntok

        token = cs("onesP")
        for b in range(BPC):
            token = batch(b, token)


def _build():
    nc = bacc.Bacc()
    io = {}
    io["x"] = nc.declare_dram_parameter("x", [BPC, P, FREE], F32, isOutput=False)
    io["consts"] = nc.declare_dram_parameter("consts", [P, CF], F32,
                                             isOutput=False)
    io["y"] = nc.declare_dram_parameter("y", [BPC, P, FREE], F32, isOutput=True)
    with tile.TileContext(nc) as tc:
        _emit(nc, tc, io)
    nc.finalize()   # bacc lowering: splits multi-waits, act tables, etc.
    return nc


_NC_CACHE = {}


def _get_nc():
    key = (NCHUNK, NSTORE, XBUFS, NRED)
    if key not in _NC_CACHE:
        _NC_CACHE[key] = _build()
    return _NC_CACHE[key]


def _prep_in_maps(inputs):
    x = np.ascontiguousarray(np.asarray(inputs["x"], dtype=np.float32))
    assert x.shape == (B, NH, CH, H, W), x.shape
    xr = x.reshape(NCORES, BPC, P, FREE)

    def t(a):
        return np.asarray(a, dtype=np.float32).T

    def v(a):
        return np.asarray(a, dtype=np.float32)

    k = np.arange(P)
    hk, ck = k // (CH * SPLIT), (k % (CH * SPLIT)) // SPLIT
    gate = float(np.asarray(inputs["gate"], dtype=np.float32).reshape(-1)[0])
    gsig = 1.0 / (1.0 + np.exp(-gate))

    vals = {
        "cmask": ((ck[:, None] == np.arange(CH)[None, :]) / S),
        "hsel": (hk[:, None] == np.arange(NH)[None, :]),
        "b128": (ck[None, :] == np.arange(CH)[:, None]),
        "ind128": (hk[None, :] == np.arange(NH)[:, None]),
        "wq_t": t(inputs["Wq"]), "wk_t": t(inputs["Wk"]),
        "wv_t": t(inputs["Wv"]), "wo_t": t(inputs["Wo"]),
        "w1_t": t(inputs["W1"]), "w2_t": t(inputs["W2"]),
        "eye4": np.eye(NH),
        "bo": np.broadcast_to(v(inputs["bo"]), (NH, CH)),
        "g1": np.broadcast_to(v(inputs["g1"]), (NH, CH)),
        "beta1": np.broadcast_to(v(inputs["beta1"]), (NH, CH)),
        "g2": np.broadcast_to(v(inputs["g2"]), (NH, CH)),
        "beta2": np.broadcast_to(v(inputs["beta2"]), (NH, CH)),
        "b1": np.broadcast_to(v(inputs["b1"]), (NH, HID)),
        "b2": np.broadcast_to(v(inputs["b2"]), (NH, CH)),
        "gsig": np.full((NH, 1), gsig),
        "omg": np.full((NH, 1), 1.0 - gsig),
        "ones4": np.ones((NH, 1)),
        "onesP": np.ones((P, 1)),
    }
    consts = np.zeros((P, CF), dtype=np.float32)
    for name, (rows, off, cols) in _CONST_LAYOUT.items():
        consts[:rows, off:off + cols] = vals[name]
    return [dict(consts=consts, x=xr[i]) for i in range(NCORES)]


def _run(inputs, **spmd_kwargs):
    from concourse.bass_utils import run_bass_kernel_spmd

    nc = _get_nc()
    in_maps = _prep_in_maps(inputs)
    res = run_bass_kernel_spmd(nc, in_maps, list(range(NCORES)), **spmd_kwargs)
    out = np.empty((B, NH, CH, H, W), dtype=np.float32)
    ov = out.reshape(NCORES, BPC, P, FREE)
    for i in range(NCORES):
        ov[i] = res.results[i]["y"]
    return out, res


def kernel(**inputs):
    return _run(inputs)[0]


# revision 36
# speedup vs baseline: 1.1901x; 1.1901x over previous
"""Trainium2 Bass kernel for nn_CrossHeadAttention.

Computation (per batch b):
  pooled = mean(x[b], spatial)                       # (NH, CH)
  aw     = tiny transformer block on pooled          # (NH, CH)
  out[b] = x[b] * (1 + aw)[..., None, None]

Memory-bound: 256 MiB in + 256 MiB out, data-parallel over batch
(32 batches -> 8 cores x 4 batches). Per core, each batch's
(4, 8, 256, 256) slab is one [128, 16384] SBUF tile
(partition = head*32 + ch*4 + spatial_quarter).

Schedule: per batch, strictly
  [chunked loads -> spatial-sum passes -> tiny math -> multiplies -> stores]
Loads ride the sync HWDGE queue, stores the scalar HWDGE queue -- two
independent rings round-robined by the 16 SDMA engines.

Engine balance (the per-core elementwise work is 8.4M-elem reduce +
8.4M-elem multiply; DVE alone would be the pipeline bottleneck):
  - spatial sums: 8 sub-chunks/batch, alternating ACT (in-place Copy with
    accum_out) and DVE (in-place tensor_scalar x1.0 with accum_out)
  - multiplies: 1 chunk on ACT (Copy w/ scale), 3 on DVE
  - tiny-math: PE matmuls; layernorm rstd = bitcast fast-inverse-sqrt
    (int32 shift/xor/add + 2 Newton steps, max rel err 5e-6) on DVE;
    gelu via tanh form 0.5*u*(1+tanh(z)) with ACT Tanh
ACT functions used: Exp, Tanh, Copy, Square -- ALL in activation-table
set 0 (exp_and_others), so exactly one ACT_TABLE_LOAD (warmed at t=0).
(Ln would pull set 5 and re-trigger ~2.7us table switches per use: the
bass inserter maps each function to the first set containing it.)

All small parameters ship in ONE packed [128, CF] DMA on the scalar queue.
Sub-chunked sums keep any scheduler gap-filling quanta small (~1-2us), so
batch b's store-critical math is never stuck behind a 4.4us younger-batch
reduce on an in-order engine.
"""

from contextlib import ExitStack

import numpy as np

import concourse.bacc as bacc
import concourse.bass as bass
import concourse.tile as tile
from concourse import mybir

NCORES = 8
B, NH, CH = 32, 4, 8
H = W = 256
S = H * W                  # spatial elements per (b, h, c) plane
HID = 4
BPC = B // NCORES          # batches per core
P = 128                    # SBUF partitions
SPLIT = P // (NH * CH)     # spatial quarters mapped to partitions
FREE = S // SPLIT          # free-dim elements per partition (16384)
NCHUNK = 4                 # load chunks per batch (2 MiB each)
CHUNK = FREE // NCHUNK
NSTORE = 2                 # stores (= half-tiles) per batch (4 MiB each)
STCOLS = FREE // NSTORE
XBUFS = 6                  # half-batch tiles in flight (6 x 4 MiB = 24 MiB)
SCALE = CH ** -0.5
EPS = 1e-5
GK0 = float(np.sqrt(2.0 / np.pi))
GK1 = float(np.sqrt(2.0 / np.pi) * 0.044715)
MAGICP1 = 0x5F3759E0       # fast-rsqrt magic + 1
LOG2E = float(np.log2(np.e))
# cubic fit of 2^f on [-0.5, 1] (covers either f32->i32 rounding mode)
FE0, FE1, FE2, FE3 = 0.99988085, 0.6915571, 0.24245486, 0.06474001
F32 = mybir.dt.float32
I32 = mybir.dt.int32
AFT = mybir.ActivationFunctionType
ALU = mybir.AluOpType
AX = mybir.AxisListType

NRED = 8                   # ACT spatial-sum quanta per batch (2048 cols)
RCOLS = FREE // NRED

# ---- packed constant layout: name -> (rows, col_off, cols) ----
_CONST_LAYOUT = {}
_CF = 0


def _alloc_const(name, rows, cols):
    global _CF
    _CONST_LAYOUT[name] = (rows, _CF, cols)
    _CF += cols


_alloc_const("cmask", P, CH)      # [k, c] = (c(k)==c) / S
_alloc_const("hsel", P, NH)       # [k, h] = (h(k)==h)
_alloc_const("b128", CH, P)       # [c, k] = (c(k)==c)
_alloc_const("ind128", NH, P)     # [h, k] = (h(k)==h)
_alloc_const("wq_t", CH, CH)
_alloc_const("wk_t", CH, CH)
_alloc_const("wv_t", CH, CH)
_alloc_const("wo_t", CH, CH)
_alloc_const("w1_t", CH, HID)
_alloc_const("w2_t", HID, CH)
_alloc_const("eye4", NH, NH)
_alloc_const("bo", NH, CH)        # broadcast rows
_alloc_const("g1", NH, CH)
_alloc_const("beta1", NH, CH)
_alloc_const("g2", NH, CH)
_alloc_const("beta2", NH, CH)
_alloc_const("b1", NH, HID)
_alloc_const("b2", NH, CH)
_alloc_const("gsig", NH, 1)       # sigmoid(gate), host-computed
_alloc_const("omg", NH, 1)        # 1 - sigmoid(gate)
_alloc_const("ones4", NH, 1)
_alloc_const("onesP", P, 1)       # neutral x1.0 token for batch 0's sums
CF = _CF


def _emit(nc, tc, io):
    with ExitStack() as ctx:
        const = ctx.enter_context(tc.tile_pool(name="const", bufs=1))
        xp = ctx.enter_context(tc.tile_pool(name="xp", bufs=XBUFS))
        sm = ctx.enter_context(tc.tile_pool(name="sm", bufs=6))
        ps = ctx.enter_context(tc.tile_pool(name="ps", bufs=8, space="PSUM"))

        const_t = const.tile([P, CF], F32, tag="c_all")
        nc.scalar.dma_start(out=const_t, in_=io["consts"][:])

        def cs(name):
            rows, off, cols = _CONST_LAYOUT[name]
            return const_t[0:rows, off:off + cols]

        cmask, hsel, b128, ind128 = cs("cmask"), cs("hsel"), cs("b128"), cs("ind128")
        wq_t, wk_t, wv_t, wo_t = cs("wq_t"), cs("wk_t"), cs("wv_t"), cs("wo_t")
        w1_t, w2_t, eye4 = cs("w1_t"), cs("w2_t"), cs("eye4")
        bo_bc, g1_bc, beta1_bc = cs("bo"), cs("g1"), cs("beta1")
        g2_bc, beta2_bc = cs("g2"), cs("beta2")
        b1_bc, b2_bc = cs("b1"), cs("b2")
        gsig4, omg4 = cs("gsig"), cs("omg")

        # warm the single ACT table set (exp_and_others) while batch 0 loads
        warm = sm.tile([1, 1], F32, tag="warm")
        nc.vector.memset(warm, 0.0)
        warm2 = sm.tile([1, 1], F32, tag="warm2")
        nc.scalar.activation(out=warm2, in_=warm, func=AFT.Exp)

        def pe_t(src, f, tag):
            # [4, f] -> [f, 4] via PE transpose (fp32 has no DMA transpose)
            tp = ps.tile([f, NH], F32, tag="ps")
            nc.tensor.transpose(tp, src, eye4)
            t = sm.tile([f, NH], F32, tag=tag)
            nc.vector.tensor_copy(out=t, in_=tp)
            return t

        def mm(lhsT, rhs, m, n, tag=None):
            op = ps.tile([m, n], F32, tag="ps")
            nc.tensor.matmul(op, lhsT, rhs, start=True, stop=True)
            if tag is None:
                return op
            t = sm.tile([m, n], F32, tag=tag)
            nc.vector.tensor_copy(out=t, in_=op)
            return t

        def rsqrt4(var, tag):
            # y = 1/sqrt(var + EPS), fast-inverse-sqrt on DVE: bits' =
            # ~(bits >> 1) + (MAGIC+1)  ==  MAGIC - (bits >> 1),
            # then 2 Newton steps y <- y*(1.5 - 0.5*v*y^2).  Max rel 5e-6.
            vpe = sm.tile([NH, 1], F32, tag=tag + "_v")
            nc.vector.tensor_scalar_add(out=vpe, in0=var, scalar1=EPS)
            y = sm.tile([NH, 1], F32, tag=tag + "_y")
            nc.vector.tensor_scalar(out=y[:].bitcast(I32),
                                    in0=vpe[:].bitcast(I32),
                                    scalar1=1, scalar2=-1,
                                    op0=ALU.arith_shift_right,
                                    op1=ALU.bitwise_xor)
            nc.vector.tensor_scalar_add(out=y[:].bitcast(I32),
                                        in0=y[:].bitcast(I32),
                                        scalar1=MAGICP1)
            for it in range(2):
                a = sm.tile([NH, 1], F32, tag=f"{tag}_a{it}")
                nc.vector.tensor_mul(out=a, in0=y, in1=y)
                nc.vector.tensor_mul(out=a, in0=a, in1=vpe)
                nc.vector.tensor_scalar(out=a, in0=a, scalar1=-0.5,
                                        scalar2=1.5, op0=ALU.mult, op1=ALU.add)
                yn = sm.tile([NH, 1], F32, tag=f"{tag}_y{it}")
                nc.vector.tensor_mul(out=yn, in0=y, in1=a)
                y = yn
            return y

        def layernorm(src, g_bc, b_bc, tag):
            stats = sm.tile([NH, nc.vector.BN_STATS_DIM], F32, tag=tag + "_st")
            nc.vector.bn_stats(out=stats, in_=src)
            mv = sm.tile([NH, 2], F32, tag=tag + "_mv")
            nc.vector.bn_aggr(out=mv, in_=stats)
            rstd = rsqrt4(mv[:, 1:2], tag)
            xn = sm.tile([NH, CH], F32, tag=tag + "_o")
            nc.vector.tensor_scalar(out=xn, in0=src, scalar1=mv[:, 0:1],
                                    scalar2=rstd, op0=ALU.subtract, op1=ALU.mult)
            nc.vector.tensor_mul(out=xn, in0=xn, in1=g_bc)
            nc.vector.tensor_add(out=xn, in0=xn, in1=b_bc)
            return xn

        def batch(b, token):
            # Two half-batch tiles: finer SBUF reuse (a future batch's loads
            # only wait on one 4 MiB store, not a whole 8 MiB batch), and
            # per-half stores ring as soon as that half's 2 multiplies land.
            #
            # Every spatial-sum op takes `token` (prev batch's mcol rescaled
            # to exactly 1.0) as a neutral multiplicative operand: a REAL
            # data dependency that forbids the list scheduler from placing
            # this batch's 2-4.5us sum quanta ahead of the previous batch's
            # store-critical math/multiply ops on the in-order DVE/ACT
            # engines (in v3 that inversion serialized all stores of batches
            # 1-3 behind the final loads: +30us).
            # sums live ONLY on ACT (small 2048-col quanta); math + all four
            # multiplies live ONLY on DVE (+PE).  The list scheduler then has
            # no sum quantum it could insert between the ~30 small DVE ops of
            # an older batch's store-critical math chain -- only the two tiny
            # ACT crossings (softmax Exp, gelu Tanh) can eat a <2us insertion.
            halves = []
            sums8 = sm.tile([P, NRED], F32, tag="sums8")
            for h in range(NSTORE):
                xh = xp.tile([P, STCOLS], F32, tag="xh")
                for cc in range(NCHUNK // NSTORE):
                    c = h * (NCHUNK // NSTORE) + cc
                    lsl = slice(cc * CHUNK, (cc + 1) * CHUNK)
                    nc.sync.dma_start(out=xh[:, lsl],
                                      in_=io["x"][b][:, c * CHUNK:(c + 1) * CHUNK])
                    for j in range(NRED // NCHUNK):
                        q = c * (NRED // NCHUNK) + j
                        rsl = slice(cc * CHUNK + j * RCOLS,
                                    cc * CHUNK + (j + 1) * RCOLS)
                        nc.scalar.activation(out=xh[:, rsl], in_=xh[:, rsl],
                                             func=AFT.Copy, scale=token,
                                             accum_out=sums8[:, q:q + 1])
                halves.append(xh)

            # spatial mean: fold sub-sums, then fold the partition quarters
            # into pooled [4h, 8c] via selection matmul.
            sums = sm.tile([P, 1], F32, tag="sums")
            nc.vector.reduce_sum(out=sums, in_=sums8, axis=AX.X)
            # next batch's sum-ordering token (sums * 0 + 1 == 1.0): younger
            # sum quanta may never be scheduled ahead of this batch's fold
            ntok = sm.tile([P, 1], F32, tag="ntok")
            nc.vector.tensor_scalar(out=ntok, in0=sums, scalar1=0.0,
                                    scalar2=1.0, op0=ALU.mult, op1=ALU.add)
            csums = sm.tile([P, CH], F32, tag="csums")
            nc.vector.tensor_scalar_mul(out=csums, in0=cmask, scalar1=sums)
            pooled_ps = ps.tile([NH, CH], F32, tag="ps")
            nc.tensor.matmul(pooled_ps, hsel, csums, start=True, stop=True)
            pooled = sm.tile([NH, CH], F32, tag="pooled")
            nc.vector.tensor_copy(out=pooled, in_=pooled_ps)

            xn = layernorm(pooled, g1_bc, beta1_bc, "ln1")
            xnT = pe_t(xn, CH, "xnT")                    # [8, 4]
            qT = mm(wq_t, xnT, CH, NH, "qT")             # [8, 4] = Wq @ xn.T
            kT = mm(wk_t, xnT, CH, NH, "kT")
            v = mm(xnT, wv_t, NH, CH, "v")               # [4, 8] = xn @ Wv.T
            sc = mm(qT, kT, NH, NH)                      # psum [4h, 4g]
            es = sm.tile([NH, NH], F32, tag="es")
            nc.scalar.activation(out=es, in_=sc, func=AFT.Exp, scale=SCALE)
            rs = sm.tile([NH, 1], F32, tag="rs")
            nc.vector.reduce_sum(out=rs, in_=es, axis=AX.X)
            rr = sm.tile([NH, 1], F32, tag="rr")
            nc.vector.reciprocal(out=rr, in_=rs)
            attn = sm.tile([NH, NH], F32, tag="attn")
            nc.vector.tensor_scalar_mul(out=attn, in0=es, scalar1=rr)
            attnT = pe_t(attn, NH, "attnT")              # [4g, 4h]
            ao = mm(attnT, v, NH, CH, "ao")              # [4, 8] = attn @ V
            aoT = pe_t(ao, CH, "aoT")                    # [8, 4]
            o_ps = mm(aoT, wo_t, NH, CH)                 # psum [4, 8] = ao @ Wo.T
            xat = sm.tile([NH, CH], F32, tag="xat")
            nc.vector.tensor_add(out=xat, in0=o_ps, in1=bo_bc)
            nc.vector.tensor_add(out=xat, in0=xat, in1=pooled)

            xn2 = layernorm(xat, g2_bc, beta2_bc, "ln2")
            xn2T = pe_t(xn2, CH, "xn2T")                 # [8, 4]
            h1_ps = mm(xn2T, w1_t, NH, HID)              # psum [4, 4]
            u = sm.tile([NH, HID], F32, tag="u")
            nc.vector.tensor_add(out=u, in0=h1_ps, in1=b1_bc)
            # tanh-gelu: z = GK0*u + GK1*u^3; gelu = u * (0.5 + 0.5*tanh(z))
            usq = sm.tile([NH, HID], F32, tag="usq")
            nc.scalar.activation(out=usq, in_=u, func=AFT.Square)
            w = sm.tile([NH, HID], F32, tag="w")
            nc.vector.tensor_scalar(out=w, in0=usq, scalar1=GK1, scalar2=GK0,
                                    op0=ALU.mult, op1=ALU.add)
            z = sm.tile([NH, HID], F32, tag="z")
            nc.vector.tensor_mul(out=z, in0=w, in1=u)
            th = sm.tile([NH, HID], F32, tag="th")
            nc.scalar.activation(out=th, in_=z, func=AFT.Tanh)
            hf = sm.tile([NH, HID], F32, tag="hf")
            nc.vector.tensor_scalar(out=hf, in0=th, scalar1=0.5, scalar2=0.5,
                                    op0=ALU.mult, op1=ALU.add)
            h1g = sm.tile([NH, HID], F32, tag="h1g")
            nc.vector.tensor_mul(out=h1g, in0=u, in1=hf)

            h1gT = pe_t(h1g, HID, "h1gT")                # [4hid, 4h]
            f_ps = mm(h1gT, w2_t, NH, CH)                # psum [4, 8]
            xo = sm.tile([NH, CH], F32, tag="xo")
            nc.vector.tensor_add(out=xo, in0=f_ps, in1=b2_bc)
            nc.vector.tensor_add(out=xo, in0=xo, in1=xat)

            # m = 1 + aw = (g * x_out + 1) + (1 - g) * pooled
            d = sm.tile([NH, CH], F32, tag="d")
            nc.vector.tensor_scalar(out=d, in0=xo, scalar1=gsig4,
                                    scalar2=1.0, op0=ALU.mult, op1=ALU.add)
            m4 = sm.tile([NH, CH], F32, tag="m4")
            nc.vector.scalar_tensor_tensor(out=m4, in0=pooled, scalar=omg4,
                                           in1=d, op0=ALU.mult, op1=ALU.add)

            # expand m4 [4h, 8c] -> per-partition scalar mcol [128, 1] with
            # PE only: W128[h, k] = m4[h, c(k)]; mask rows by h(k); column
            # sums distribute the selected value to every partition k.
            m4T = pe_t(m4, CH, "m4T")                    # [8c, 4h]
            w128_ps = ps.tile([NH, P], F32, tag="ps")
            nc.tensor.matmul(w128_ps, m4T, b128, start=True, stop=True)
            v128 = sm.tile([NH, P], F32, tag="v128")
            nc.vector.tensor_mul(out=v128, in0=w128_ps, in1=ind128)
            mcol_ps = ps.tile([P, 1], F32, tag="ps")
            nc.tensor.matmul(mcol_ps, v128, cs("ones4"), start=True, stop=True)
            mcol = sm.tile([P, 1], F32, tag="mcol")
            nc.vector.tensor_copy(out=mcol, in_=mcol_ps)

            # multiply in place (all DVE) + stores; the last batch stores
            # per-chunk so its tail drain starts right after the first mult
            last = b == BPC - 1
            for h in range(NSTORE):
                xh = halves[h]
                for cc in range(NCHUNK // NSTORE):
                    lsl = slice(cc * CHUNK, (cc + 1) * CHUNK)
                    nc.vector.tensor_scalar_mul(out=xh[:, lsl],
                                                in0=xh[:, lsl], scalar1=mcol)
                    if last:
                        c = h * (NCHUNK // NSTORE) + cc
                        nc.scalar.dma_start(
                            out=io["y"][b][:, c * CHUNK:(c + 1) * CHUNK],
                            in_=xh[:, lsl])
                if not last:
                    ssl = slice(h * STCOLS, (h + 1) * STCOLS)
                    nc.scalar.dma_start(out=io["y"][b][:, ssl], in_=xh)
            return ntok

        token = cs("onesP")
        for b in range(BPC):
            token = batch(b, token)


def _build():
    nc = bacc.Bacc()
    io = {}
    io["x"] = nc.declare_dram_parameter("x", [BPC, P, FREE], F32, isOutput=False)
    io["consts"] = nc.declare_dram_parameter("consts", [P, CF], F32,
                                             isOutput=False)
    io["y"] = nc.declare_dram_parameter("y", [BPC, P, FREE], F32, isOutput=True)
    with tile.TileContext(nc) as tc:
        _emit(nc, tc, io)
    nc.finalize()   # bacc lowering: splits multi-waits, act tables, etc.
    return nc


_NC_CACHE = {}


def _get_nc():
    key = (NCHUNK, NSTORE, XBUFS, NRED)
    if key not in _NC_CACHE:
        _NC_CACHE[key] = _build()
    return _NC_CACHE[key]


def _prep_in_maps(inputs):
    x = np.ascontiguousarray(np.asarray(inputs["x"], dtype=np.float32))
    assert x.shape == (B, NH, CH, H, W), x.shape
    xr = x.reshape(NCORES, BPC, P, FREE)

    def t(a):
        return np.asarray(a, dtype=np.float32).T

    def v(a):
        return np.asarray(a, dtype=np.float32)

    k = np.arange(P)
    hk, ck = k // (CH * SPLIT), (k % (CH * SPLIT)) // SPLIT
    gate = float(np.asarray(inputs["gate"], dtype=np.float32).reshape(-1)[0])
    gsig = 1.0 / (1.0 + np.exp(-gate))

    vals = {
        "cmask": ((ck[:, None] == np.arange(CH)[None, :]) / S),
        "hsel": (hk[:, None] == np.arange(NH)[None, :]),
        "b128": (ck[None, :] == np.arange(CH)[:, None]),
        "ind128": (hk[None, :] == np.arange(NH)[:, None]),
        "wq_t": t(inputs["Wq"]), "wk_t": t(inputs["Wk"]),
        "wv_t": t(inputs["Wv"]), "wo_t": t(inputs["Wo"]),
        "w1_t": t(inputs["W1"]), "w2_t": t(inputs["W2"]),
        "eye4": np.eye(NH),
        "bo": np.broadcast_to(v(inputs["bo"]), (NH, CH)),
        "g1": np.broadcast_to(v(inputs["g1"]), (NH, CH)),
        "beta1": np.broadcast_to(v(inputs["beta1"]), (NH, CH)),
        "g2": np.broadcast_to(v(inputs["g2"]), (NH, CH)),
        "beta2": np.broadcast_to(v(inputs["beta2"]), (NH, CH)),
        "b1": np.broadcast_to(v(inputs["b1"]), (NH, HID)),
        "b2": np.broadcast_to(v(inputs["b2"]), (NH, CH)),
        "gsig": np.full((NH, 1), gsig),
        "omg": np.full((NH, 1), 1.0 - gsig),
        "ones4": np.ones((NH, 1)),
        "onesP": np.ones((P, 1)),
    }
    consts = np.zeros((P, CF), dtype=np.float32)
    for name, (rows, off, cols) in _CONST_LAYOUT.items():
        consts[:rows, off:off + cols] = vals[name]
    return [dict(consts=consts, x=xr[i]) for i in range(NCORES)]


def _run(inputs, **spmd_kwargs):
    from concourse.bass_utils import run_bass_kernel_spmd

    nc = _get_nc()
    in_maps = _prep_in_maps(inputs)
    res = run_bass_kernel_spmd(nc, in_maps, list(range(NCORES)), **spmd_kwargs)
    out = np.empty((B, NH, CH, H, W), dtype=np.float32)
    ov = out.reshape(NCORES, BPC, P, FREE)
    for i in range(NCORES):
        ov[i] = res.results[i]["y"]
    return out, res


def kernel(**inputs):
    return _run(inputs)[0]
